# revision 87
# baseline (speedup 1.0000x reference)
"""Multi-head attention forward on 8 TRN2 NeuronCores (data-parallel over batch).

Reference computation (B=64, T=197, D=768, H=12, DK=64, fp32):
    q = split_heads(x @ Wq + bq); k = ...; v = ...
    scores = floor((q @ k^T) / 8); attn = softmax(scores); out = attn @ v
    return merge_heads(out) @ Wo + bo

Numerics:
  * q/k projections: fp16 2-term matmuls (x split exactly as
    x_hi + x_lo; W rounded to fp16): q = (x_hi + x_lo) @ fp16(W) with
    fp32 PSUM accumulation.  Only W's fp16 rounding (~5e-4 relative)
    reaches the floor(), measured ~1.2e-2 final relative error vs the
    2e-2 gate.  Set THREE_TERM=True for the exact 3-term scheme
    (a_hi@b_hi + a_hi@b_lo + a_lo@b_hi, ~1.3e-3, +50us).
  * scores: fp16 (qT/kT stored fp16, 1 cyc/row), 2 heads row-packed
    via tile_position; fp32 PSUM accumulation.
  * v path (v proj, attn@v, out proj): plain fp16.
  * floor via round-half-even magic-number add (DVE, magic 1.5*2^23)
    with the -magic correction folded into the ScalarE Exp bias.
  * attn@v numerators kept in fp16 (values < ~2^13, well inside range).

Schedule (the change vs the 456us baseline): a fused pipeline over
column chunks of CW=394 tokens = 2 batch elements.  Each engine
executes the Tile scheduler's STATIC order, so latency-bound attention
work (scores -> DVE floor -> ScalarE exp -> attn@v -> normalize ->
out-proj) is emitted in ~0.5us units MANUALLY INTERLEAVED with dense
next-chunk work (x DMA+PE-transpose+split, q/k/v projections).  The PE
then always has a ready matmul between dependent attention steps: no
idle gaps, and the HAM clock gate stays at full rate.  Weights are
host-prearranged to [128, 6*768] so each matrix is ONE contiguous DMA
(sync-engine issue is ~0.6us per dma_start); x rowchunk DMAs are
issued first so transposes start immediately.  qT/kT live per-chunk
(bufs=2) so SBUF fits.  The last chunk (no next-chunk filler) gets the
final batch element's deferred v-projection hand-placed in its chain.

Measured: 272.4us HW exec (vs 456us baseline, 1.67x; 326us under P0
thermal downclock -- expect ~±20% run-to-run from the chip's power
state under sustained benching load).  Rel err 1.575e-2
(deterministic; gate 2e-2).  PE 88-89% busy, schedule gaps ~15us.

Bias matmuls (K=1 ones-row) are only emitted when any bias is nonzero;
the build is specialized on that flag.  All PSUM tiles come from one
shared-tag pool (8 banks round-robin) so phases overlap freely.
"""

import numpy as np
import ml_dtypes

B, T, D, H, DK = 64, 197, 768, 12, 64
NCORES = 8
BL = B // NCORES          # 8 batch elements per core
R = BL * T                # 1576 rows per core
ND = D // 128             # 6 chunks of 128 along D
NC4 = 4                   # proj col chunks (each = 2 batch elements)
CW = R // NC4             # 394
HV = DK + 1               # 65: per-head v stride (ones column at 64)
ROWCHUNKS = [(i * 128, min(128, R - i * 128)) for i in range((R + 127) // 128)]
KEYCHUNKS = [(0, 128), (128, 69)]
MAGIC = float(3 * 2 ** 22)  # 1.5*2^23: x-0.5+MAGIC stays in [2^23,2^24), ulp=1
THREE_TERM = False        # q/k proj: 3-term exact fp16 vs 2-term (W fp16 RN)

_CACHE = {}


def _build(has_bias):
    import concourse.bacc as bacc
    import concourse.mybir as mybir
    import concourse.tile as tile
    from concourse.masks import make_identity

    f32 = mybir.dt.float32
    f16 = mybir.dt.float16
    AF = mybir.ActivationFunctionType
    OP = mybir.AluOpType

    nc = bacc.Bacc("TRN2", target_bir_lowering=False, debug=False,
                   num_devices=NCORES)

    wnames = ("wq_hi", "wq_lo", "wk_hi", "wk_lo", "wv", "wo") if THREE_TERM \
        else ("wq_hi", "wk_hi", "wv", "wo")
    x_d = nc.dram_tensor("x", [R, D], f32, kind="ExternalInput").ap()
    w_d = {}
    for nm in wnames:
        # host-prearranged: [128, ND*D]; col block k = W rows k*128..+127
        w_d[nm] = nc.dram_tensor(nm, [128, ND * D], f16,
                                 kind="ExternalInput").ap()
    if has_bias:
        bq_d = nc.dram_tensor("bq", [1, D], f16, kind="ExternalInput").ap()
        bk_d = nc.dram_tensor("bk", [1, D], f16, kind="ExternalInput").ap()
        bv_d = nc.dram_tensor("bv", [1, D], f16, kind="ExternalInput").ap()
        bo_d = nc.dram_tensor("bo", [1, D], f16, kind="ExternalInput").ap()
    out_d = nc.dram_tensor("out", [R, D], f32, kind="ExternalOutput").ap()

    with tile.TileContext(nc) as tc:
        with tc.tile_pool(name="static", bufs=1) as Ps, \
             tc.tile_pool(name="work", bufs=1) as Pw, \
             tc.tile_pool(name="psum", bufs=8, space="PSUM") as Pp:

            def ptile(nm):
                return Pp.tile([128, CW], f32, name=nm, tag="ps", bufs=8,
                               uniquify=True)

            # ---------------- resident tiles ----------------
            wsb = {nm: Ps.tile([128, ND * D], f16, name=nm) for nm in wnames}

            def wslice(nm, k, cols):
                # wv/wo: by-k layout (col block k = W rows k*128..+127)
                return wsb[nm][:, k * D + cols.start:k * D + cols.stop]

            def wslice_qk(nm, k, n):
                # wq_hi/wk_hi: by-n layout (col block n holds all k slices
                # of output block n) so projections for n=0 can start after
                # 1/6 of the weight DMA
                return wsb[nm][:, n * D + k * 128:n * D + (k + 1) * 128]

            xhi = [Ps.tile([128, R], f16, name=f"xhi{i}") for i in range(ND)]
            xlo = [Ps.tile([128, R], f16, name=f"xlo{i}") for i in range(ND)]
            ones_row = Ps.tile([128, CW], f16, name="ones_row")
            id32 = Ps.tile([128, 128], f32, name="id32")
            negmagic = Ps.tile([128, 1], f32, name="negmagic")

            nc.vector.memset(ones_row, 1.0)
            nc.vector.memset(negmagic, -MAGIC)
            # HAM warm-up: ~5us of dummy matmuls while the x/weight DMAs
            # are in flight, so the PE clock gate is at full rate (8/8)
            # by the time real work issues.  Results are never read.
            for _ in range(16):
                wt = ptile("warm")
                nc.tensor.matmul(wt, ones_row[:, :128], ones_row,
                                 start=True, stop=True)
            make_identity(nc, id32)
            if has_bias:
                bq_sb = Ps.tile([1, D], f16, name="bq_sb")
                bk_sb = Ps.tile([1, D], f16, name="bk_sb")
                bv_sb = Ps.tile([1, D], f16, name="bv_sb")
                bo_sb = Ps.tile([1, D], f16, name="bo_sb")

            # ---------------- pipeline pieces ----------------
            def load_split_rowchunk(rc):
                roff, rn = ROWCHUNKS[rc]
                xs = Pw.tile([128, D], f32, name="xs", tag="xs", bufs=5)
                nc.sync.dma_start(xs[:rn, :], x_d[roff:roff + rn, :])
                for d in range(ND):
                    tp = ptile("tp")
                    nc.tensor.transpose(tp[:128, :rn],
                                        xs[:rn, d * 128:(d + 1) * 128],
                                        id32[:rn, :rn])
                    hi = xhi[d][:, roff:roff + rn]
                    nc.vector.tensor_copy(hi, tp[:128, :rn])
                    nc.vector.tensor_tensor(xlo[d][:, roff:roff + rn],
                                            tp[:128, :rn], hi, OP.subtract)

            def proj_group(pre, n, c, dst_tiles):
                """One q/k projection group: 18 fp16 MMs -> qT/kT tile."""
                whi, wlo, b_nm = {"q": ("wq_hi", "wq_lo", "bq"),
                                  "k": ("wk_hi", "wk_lo", "bk")}[pre]
                cs = slice(c * CW, (c + 1) * CW)
                ns = slice(n * 128, (n + 1) * 128)
                # fp16 qT/kT: the scores matmuls then run at 1 cyc/row
                # (4x cheaper than fp32).  Emulated+HW-validated: adds
                # q/k fp16 rounding ahead of the floor(), total rel err
                # ~1.5e-2 vs the 2e-2 gate.
                dst = Pw.tile([128, CW], f16, name=f"{pre}T{n}",
                              tag=f"{pre}T{n}", bufs=2)
                pp = ptile("pp")
                for k in range(ND):
                    last = k == ND - 1 and not has_bias
                    nc.tensor.matmul(pp, wslice_qk(whi, k, n), xhi[k][:, cs],
                                     start=(k == 0), stop=False)
                    nc.tensor.matmul(pp, wslice_qk(whi, k, n), xlo[k][:, cs],
                                     start=False,
                                     stop=(last and not THREE_TERM))
                    if THREE_TERM:
                        nc.tensor.matmul(pp, wslice_qk(wlo, k, n),
                                         xhi[k][:, cs],
                                         start=False, stop=last)
                if has_bias:
                    bsb = {"bq": bq_sb, "bk": bk_sb}[b_nm]
                    nc.tensor.matmul(pp, bsb[:1, ns], ones_row[:1, :CW],
                                     start=False, stop=True)
                nc.scalar.activation(dst, pp, AF.Copy)
                dst_tiles[n] = dst

            def vproj_kc(b, kc, dst_tiles):
                """v projection for batch b, key chunk kc (fp16)."""
                base = b * T
                koff, klen = KEYCHUNKS[kc]
                dst = Pw.tile([128, H * HV], f16, name=f"v16e_{b}_{kc}",
                              tag="v16e", bufs=8)
                # only the per-head ones-columns need initializing; the
                # value region is fully overwritten by the copies below
                nc.vector.memset(
                    dst[:klen, :].rearrange(
                        "p (h c) -> p h c", c=HV)[:, :, DK:HV], 1.0)
                dst3 = dst[:klen, :].rearrange(
                    "p (h c) -> p h c", c=HV)[:, :, 0:DK]
                for half in range(2):
                    c0 = half * 384
                    vp = ptile("vp")
                    vps = vp[:klen, :384]
                    for d in range(ND):
                        nc.tensor.matmul(
                            vps, xhi[d][:, base + koff:base + koff + klen],
                            wslice("wv", d, slice(c0, c0 + 384)),
                            start=(d == 0),
                            stop=(d == ND - 1 and not has_bias))
                    if has_bias:
                        nc.tensor.matmul(vps, ones_row[:1, :klen],
                                         bv_sb[:1, c0:c0 + 384],
                                         start=False, stop=True)
                    nc.scalar.activation(
                        dst3[:, half * 6:(half + 1) * 6, :],
                        vps.rearrange("p (h c) -> p h c", c=DK),
                        AF.Copy)
                dst_tiles[kc] = dst

            def scores_hp(qk, bl, hp, eTs):
                """scoresT + floor + exp for one head-pair of local batch
                element bl (0/1) of the current chunk."""
                qT, kT = qk["q"], qk["k"]
                qs = slice(bl * T, (bl + 1) * T)
                eT = []
                for hl in range(2):
                    pb = 64 * hl
                    sc = ptile("sc")
                    for kc, (koff, klen) in enumerate(KEYCHUNKS):
                        ks = slice(bl * T + koff, bl * T + koff + klen)
                        nc.tensor.matmul(
                            sc[:klen, kc * T:(kc + 1) * T],
                            kT[hp][pb:pb + 64, ks],
                            qT[hp][pb:pb + 64, qs],
                            start=True, stop=True,
                            tile_position=(pb, 0))
                    fl = Pw.tile([128, 2 * T], f32, name="fl", tag="fl",
                                 bufs=6)
                    nc.vector.tensor_scalar(
                        fl, sc, -0.5, MAGIC, OP.add, OP.add)
                    e_t = Pw.tile([128, 2 * T], f16, name="e_t",
                                  tag="eT", bufs=28)
                    nc.scalar.activation(
                        e_t, fl, AF.Exp, bias=negmagic[:, :1])
                    eT.append(e_t)
                eTs[hp] = eT

            def av_hp(v16e, eTs, b, hp, st):
                """attn @ v for one head-pair; col 64 of v16e = ones ->
                denominators gathered into st['dn']."""
                if "dn" not in st:
                    st["dn"] = Pw.tile([128, 4 * T], f32, name="dn",
                                       tag="dn", bufs=2)
                    st["oT16"] = [None] * ND
                    st["otf"] = [None] * ND
                dn = st["dn"]
                otf = Pw.tile([128, T], f16, name="otf", tag="otf", bufs=13)
                op_ = ptile("oT")
                for hl in range(2):
                    h = 2 * hp + hl
                    for kc, (koff, klen) in enumerate(KEYCHUNKS):
                        nc.tensor.matmul(
                            op_[0:HV, hl * T:(hl + 1) * T],
                            v16e[kc][:klen, h * HV:(h + 1) * HV],
                            eTs[hp][hl][:klen, kc * T:(kc + 1) * T],
                            start=(kc == 0),
                            stop=(kc == len(KEYCHUNKS) - 1))
                    if hl == 0:
                        nc.scalar.activation(otf[0:64, :],
                                             op_[0:64, :T], AF.Copy)
                    else:
                        nc.vector.tensor_copy(otf[64:128, :],
                                              op_[0:64, T:2 * T])
                # denominators for both heads of the pair in one strip:
                # head-pair hp at partition 32*(hp%4), col block hp//4
                pbase = 32 * (hp % 4)
                cb = (hp // 4) * 2 * T
                nc.vector.tensor_copy(dn[pbase:pbase + 1, cb:cb + 2 * T],
                                      op_[64:65, :])
                st["otf"][hp] = otf

            def norm_recip(st):
                rdf = Pw.tile([128, 4 * T], f32, name="rdf", tag="rdf",
                              bufs=2)
                rd16 = Pw.tile([128, 4 * T], f16, name="rd16", tag="rd16",
                               bufs=2)
                nc.vector.reciprocal_approx_fast(rdf, st["dn"])
                nc.vector.tensor_copy(rd16, rdf)
                st["rd16"] = rd16

            def norm_hp(b, hp, st):
                rd16 = st["rd16"]
                oT16 = Pw.tile([128, T], f16, name=f"oT16_{b}_{hp}",
                               tag="oT16", bufs=26)
                bc = ptile("bc")
                pbase = 32 * (hp % 4)
                for hl in range(2):
                    cb = (hp // 4) * 2 * T + hl * T
                    nc.tensor.matmul(
                        bc[64 * hl:64 * hl + 64, :T],
                        ones_row[pbase:pbase + 1, :64],
                        rd16[pbase:pbase + 1, cb:cb + T],
                        start=True, stop=True,
                        tile_position=(pbase, 64 * hl))
                nc.vector.tensor_tensor(oT16, st["otf"][hp],
                                        bc[:, :T], OP.mult)
                st["oT16"][hp] = oT16

            def final_rc(b, rcl, st):
                """out-projection for token rows rcl of batch b + store."""
                base = b * T
                roff, rn = ((0, 128), (128, T - 128))[rcl]
                oT16 = st["oT16"]
                fs = Pw.tile([128, D], f32, name="fs", tag="fs", bufs=4)
                for half in range(2):
                    c0 = half * 384
                    fp_ = ptile("fp")
                    for d in range(ND):
                        nc.tensor.matmul(
                            fp_[:rn, :384],
                            oT16[d][:, roff:roff + rn],
                            wslice("wo", d, slice(c0, c0 + 384)),
                            start=(d == 0),
                            stop=(d == ND - 1 and not has_bias))
                    if has_bias:
                        nc.tensor.matmul(
                            fp_[:rn, :384], ones_row[:1, :rn],
                            bo_sb[:1, c0:c0 + 384],
                            start=False, stop=True)
                    nc.scalar.activation(fs[:rn, c0:c0 + 384],
                                         fp_[:rn, :384], AF.Copy)
                nc.sync.dma_start(
                    out_d[base + roff:base + roff + rn, :], fs[:rn, :])

            def chunk_rowchunks(c):
                lo = (CW * c) // 128
                hi = (CW * c + CW - 1) // 128
                return lo, hi

            def emit_interleaved(chain, filler):
                """Alternate chain units with filler units so the static
                per-engine order has dense PE work between dependent
                attention steps.  chain entries are (thunk, weight):
                weight = how much filler to place after the unit, scaled
                so all filler is spent.  The recip units carry extra
                weight to cover the serial DVE recip->cast chain before
                the broadcast matmuls."""
                if not chain:
                    for th in filler:
                        th()
                    return
                total_w = sum(w for _, w in chain) or 1.0
                scale = len(filler) / total_w
                acc, fi = 0.0, 0
                for th, w in chain:
                    th()
                    acc += w * scale
                    while fi < len(filler) and acc >= 1.0 - 1e-9:
                        filler[fi]()
                        fi += 1
                        acc -= 1.0
                while fi < len(filler):
                    filler[fi]()
                    fi += 1

            # ---------------- emission ----------------
            # prologue: x rowchunk DMAs+transposes for chunk 0 first, then
            # weight DMAs (q/k first), then chunk-0 projections + v.
            lo, hi = chunk_rowchunks(0)
            for rc in range(lo, hi + 1):
                load_split_rowchunk(rc)
            done_rc = hi
            whalf = ND * D // 2
            qk_first = [nm for nm in wnames if nm not in ("wv", "wo")]
            for nm in qk_first:
                nc.sync.dma_start(wsb[nm][:, :whalf], w_d[nm][:, :whalf])
            for nm in qk_first:
                nc.sync.dma_start(wsb[nm][:, whalf:], w_d[nm][:, whalf:])
            for nm in ("wv", "wo"):
                nc.sync.dma_start(wsb[nm][:, :whalf], w_d[nm][:, :whalf])
                nc.sync.dma_start(wsb[nm][:, whalf:], w_d[nm][:, whalf:])
            if has_bias:
                nc.sync.dma_start(bq_sb, bq_d)
                nc.sync.dma_start(bk_sb, bk_d)
                nc.sync.dma_start(bv_sb, bv_d)
                nc.sync.dma_start(bo_sb, bo_d)

            qk = {"q": [None] * ND, "k": [None] * ND}
            for n in range(ND):
                proj_group("q", n, 0, qk["q"])
                proj_group("k", n, 0, qk["k"])
            vt = {}
            for b in (0, 1):
                vt[b] = [None, None]
                vproj_kc(b, 0, vt[b])
                vproj_kc(b, 1, vt[b])

            deferred = []
            carry = []
            for c in range(NC4):
                b0, b1 = 2 * c, 2 * c + 1
                cur_qk, cur_v0, cur_v1 = qk, vt[b0], vt[b1]
                e0, e1 = [None] * ND, [None] * ND
                st0, st1 = {}, {}

                chain = []
                for hp in range(ND):
                    chain.append((lambda hp=hp: scores_hp(cur_qk, 0, hp, e0),
                                  1.3))
                    chain.append((lambda hp=hp: scores_hp(cur_qk, 1, hp, e1),
                                  1.3))
                # previous chunk's output projections are dependency-free
                # dense PE anchors: two placed mid-attn@v (which otherwise
                # paces behind ScalarE exp), two after the recips to cover
                # the serial DVE recip->cast chain
                for hp in range(ND):
                    chain.append((lambda hp=hp: av_hp(cur_v0, e0, b0, hp,
                                                      st0), 0.9))
                    chain.append((lambda hp=hp: av_hp(cur_v1, e1, b1, hp,
                                                      st1), 0.9))
                    if hp == 2 and len(carry) > 2:
                        chain.append((carry[0], 0.3))
                        chain.append((carry[1], 0.3))
                chain.append((lambda: norm_recip(st0), 3.0))
                chain.append((lambda: norm_recip(st1), 3.0))
                for th in carry[2:]:
                    chain.append((th, 0.3))
                for hp in range(ND):
                    chain.append((lambda hp=hp: norm_hp(b0, hp, st0), 0.8))
                    chain.append((lambda hp=hp: norm_hp(b1, hp, st1), 0.8))
                carry = []
                for rcl in range(2):
                    carry.append(lambda rcl=rcl, b=b0, st=st0:
                                 final_rc(b, rcl, st))
                    carry.append(lambda rcl=rcl, b=b1, st=st1:
                                 final_rc(b, rcl, st))
                if c == NC4 - 1:
                    for th in carry:
                        chain.append((th, 0.3))
                    carry = []

                filler = []
                if c + 1 < NC4:
                    lo, hi = chunk_rowchunks(c + 1)
                    for rc in range(done_rc + 1, hi + 1):
                        filler.append(
                            lambda rc=rc: load_split_rowchunk(rc))
                    done_rc = hi
                    nqk = {"q": [None] * ND, "k": [None] * ND}
                    for n in range(ND):
                        if c + 1 == NC4 - 1 and n == ND - 1:
                            # defer the last projection groups into the
                            # final (otherwise filler-less) chain; scores
                            # for head-pair 5 only run ~10 units in
                            deferred.append(
                                lambda n=n, c=c: proj_group("q", n, c + 1,
                                                            nqk["q"]))
                            deferred.append(
                                lambda n=n, c=c: proj_group("k", n, c + 1,
                                                            nqk["k"]))
                            continue
                        filler.append(
                            lambda n=n: proj_group("q", n, c + 1, nqk["q"]))
                        filler.append(
                            lambda n=n: proj_group("k", n, c + 1, nqk["k"]))
                    nvt = {}
                    for b in (2 * c + 2, 2 * c + 3):
                        nvt[b] = [None, None]
                        for kc in range(2):
                            if c + 1 == NC4 - 1 and b == 2 * c + 3:
                                # defer the last batch element's v proj to
                                # serve as PE filler inside the final
                                # (otherwise filler-less) chain
                                deferred.append(
                                    lambda b=b, kc=kc:
                                    vproj_kc(b, kc, nvt[b]))
                            else:
                                filler.append(
                                    lambda b=b, kc=kc:
                                    vproj_kc(b, kc, nvt[b]))
                    qk, vt = nqk, nvt

                if c == NC4 - 1 and deferred:
                    # hand-placed: deferred proj/v-proj units early in the
                    # chain (each completes before its consumer unit)
                    for i, (th, _) in enumerate(chain):
                        th()
                        if i in (1, 3, 5, 7) and (i - 1) // 2 < len(deferred):
                            deferred[(i - 1) // 2]()
                else:
                    emit_interleaved(chain, filler)

    nc.compile()
    return nc


def _split16(a):
    hi = a.astype(np.float16)
    lo = (a - hi.astype(np.float32)).astype(np.float16)
    return hi, lo


def _rearr(w16):
    """[768, 768] -> [128, 6*768]: col block k holds W rows k*128..+127."""
    return np.ascontiguousarray(
        w16.reshape(ND, 128, D).transpose(1, 0, 2).reshape(128, ND * D))


def _rearr_qk(w16):
    """[768, 768] -> [128, 6*768]: col block n holds all k-slices of
    output block n (dst[p, n*768+k*128+c] = W[k*128+p, n*128+c])."""
    return np.ascontiguousarray(
        w16.reshape(ND, 128, ND, 128).transpose(1, 2, 0, 3)
        .reshape(128, ND * D))


def _prep_weights(Wq, bq, Wk, bk, Wv, bv, Wo, bo, has_bias):
    f32 = np.float32
    wq = np.asarray(Wq, f32) * f32(0.125)
    wk = np.asarray(Wk, f32)
    wq_hi, wq_lo = _split16(wq)
    wk_hi, wk_lo = _split16(wk)
    w = {
        "wq_hi": _rearr_qk(wq_hi), "wk_hi": _rearr_qk(wk_hi),
        "wv": _rearr(np.asarray(Wv, f32).astype(np.float16)),
        "wo": _rearr(np.asarray(Wo, f32).astype(np.float16)),
    }
    if THREE_TERM:
        w["wq_lo"] = _rearr_qk(wq_lo)
        w["wk_lo"] = _rearr_qk(wk_lo)
    if has_bias:
        w["bq"] = (np.asarray(bq, f32) * f32(0.125)).astype(
            np.float16).reshape(1, D)
        w["bk"] = np.asarray(bk, f32).astype(np.float16).reshape(1, D)
        w["bv"] = np.asarray(bv, f32).astype(np.float16).reshape(1, D)
        w["bo"] = np.asarray(bo, f32).astype(np.float16).reshape(1, D)
    return w


def kernel(x, Wq, bq, Wk, bk, Wv, bv, Wo, bo):
    from concourse import bass_utils

    has_bias = any(float(np.abs(np.asarray(v)).max()) != 0.0
                   for v in (bq, bk, bv, bo))
    key = ("nc", has_bias)
    if key not in _CACHE:
        _CACHE[key] = _build(has_bias)
    nc = _CACHE[key]

    x = np.asarray(x, np.float32)
    w = _prep_weights(Wq, bq, Wk, bk, Wv, bv, Wo, bo, has_bias)
    in_maps = []
    for c in range(NCORES):
        m = dict(w)
        m["x"] = np.ascontiguousarray(
            x[c * BL:(c + 1) * BL].reshape(R, D))
        in_maps.append(m)

    res = bass_utils.run_bass_kernel_spmd(nc, in_maps, list(range(NCORES)))
    out = np.concatenate(
        [res.results[c]["out"].reshape(BL, T, D) for c in range(NCORES)],
        axis=0)
    return out.astype(np.float32)


# revision 88
# speedup vs baseline: 1.2082x; 1.2082x over previous
"""Multi-head attention forward on 8 TRN2 NeuronCores (data-parallel over batch).

Reference computation (B=64, T=197, D=768, H=12, DK=64, fp32):
    q = split_heads(x @ Wq + bq); k = ...; v = ...
    scores = floor((q @ k^T) / 8); attn = softmax(scores); out = attn @ v
    return merge_heads(out) @ Wo + bo

Numerics:
  * q/k projections: fp16 2-term matmuls (x split exactly as
    x_hi + x_lo; W rounded to fp16): q = (x_hi + x_lo) @ fp16(W) with
    fp32 PSUM accumulation.  Only W's fp16 rounding (~5e-4 relative)
    reaches the floor(), measured ~1.2e-2 final relative error vs the
    2e-2 gate.  Set THREE_TERM=True for the exact 3-term scheme
    (a_hi@b_hi + a_hi@b_lo + a_lo@b_hi, ~1.3e-3, +50us).
  * scores: fp16 (qT/kT stored fp16, 1 cyc/row), 2 heads row-packed
    via tile_position; fp32 PSUM accumulation.
  * v path (v proj, attn@v, out proj): plain fp16.
  * floor via round-half-even magic-number add (DVE, magic 1.5*2^23)
    with the -magic correction folded into the ScalarE Exp bias.
  * attn@v numerators kept in fp16 (values < ~2^13, well inside range).

Schedule (the change vs the 456us baseline): a fused pipeline over
column chunks of CW=394 tokens = 2 batch elements.  Each engine
executes the Tile scheduler's STATIC order, so latency-bound attention
work (scores -> DVE floor -> ScalarE exp -> attn@v -> normalize ->
out-proj) is emitted in ~0.5us units MANUALLY INTERLEAVED with dense
next-chunk work (x DMA+PE-transpose+split, q/k/v projections).  The PE
then always has a ready matmul between dependent attention steps: no
idle gaps, and the HAM clock gate stays at full rate.  Weights are
host-prearranged to [128, 6*768] so each matrix is ONE contiguous DMA
(sync-engine issue is ~0.6us per dma_start); x rowchunk DMAs are
issued first so transposes start immediately.  qT/kT live per-chunk
(bufs=2) so SBUF fits.  The last chunk (no next-chunk filler) gets the
final batch element's deferred v-projection hand-placed in its chain.

Measured: 272.4us HW exec (vs 456us baseline, 1.67x; 326us under P0
thermal downclock -- expect ~±20% run-to-run from the chip's power
state under sustained benching load).  Rel err 1.575e-2
(deterministic; gate 2e-2).  PE 88-89% busy, schedule gaps ~15us.

Bias matmuls (K=1 ones-row) are only emitted when any bias is nonzero;
the build is specialized on that flag.  All PSUM tiles come from one
shared-tag pool (8 banks round-robin) so phases overlap freely.
"""

import numpy as np
import ml_dtypes

B, T, D, H, DK = 64, 197, 768, 12, 64
NCORES = 8
BL = B // NCORES          # 8 batch elements per core
R = BL * T                # 1576 rows per core
ND = D // 128             # 6 chunks of 128 along D
NC4 = 4                   # proj col chunks (each = 2 batch elements)
CW = R // NC4             # 394
HV = DK + 1               # 65: per-head v stride (ones column at 64)
ROWCHUNKS = [(i * 128, min(128, R - i * 128)) for i in range((R + 127) // 128)]
KEYCHUNKS = [(0, 128), (128, 69)]
MAGIC = float(3 * 2 ** 22)  # 1.5*2^23: x-0.5+MAGIC stays in [2^23,2^24), ulp=1
THREE_TERM = False        # q/k proj: 3-term exact fp16 vs 2-term (W fp16 RN)

_CACHE = {}


def _build(has_bias):
    import concourse.bacc as bacc
    import concourse.mybir as mybir
    import concourse.tile as tile
    from concourse.masks import make_identity

    f32 = mybir.dt.float32
    f16 = mybir.dt.float16
    AF = mybir.ActivationFunctionType
    OP = mybir.AluOpType

    nc = bacc.Bacc("TRN2", target_bir_lowering=False, debug=False,
                   num_devices=NCORES)

    wnames = ("wq_hi", "wq_lo", "wk_hi", "wk_lo", "wv", "wo") if THREE_TERM \
        else ("wq_hi", "wk_hi", "wv", "wo")
    x_d = nc.dram_tensor("x", [R, D], f32, kind="ExternalInput").ap()
    w_d = {}
    for nm in wnames:
        # host-prearranged: [128, ND*D]; col block k = W rows k*128..+127
        w_d[nm] = nc.dram_tensor(nm, [128, ND * D], f16,
                                 kind="ExternalInput").ap()
    if has_bias:
        bq_d = nc.dram_tensor("bq", [1, D], f16, kind="ExternalInput").ap()
        bk_d = nc.dram_tensor("bk", [1, D], f16, kind="ExternalInput").ap()
        bv_d = nc.dram_tensor("bv", [1, D], f16, kind="ExternalInput").ap()
        bo_d = nc.dram_tensor("bo", [1, D], f16, kind="ExternalInput").ap()
    out_d = nc.dram_tensor("out", [R, D], f32, kind="ExternalOutput").ap()

    with tile.TileContext(nc) as tc:
        with tc.tile_pool(name="static", bufs=1) as Ps, \
             tc.tile_pool(name="work", bufs=1) as Pw, \
             tc.tile_pool(name="psum", bufs=8, space="PSUM") as Pp:

            def ptile(nm):
                return Pp.tile([128, CW], f32, name=nm, tag="ps", bufs=8,
                               uniquify=True)

            # ---------------- resident tiles ----------------
            wsb = {nm: Ps.tile([128, ND * D], f16, name=nm) for nm in wnames}

            def wslice(nm, k, cols):
                # wv/wo: by-k layout (col block k = W rows k*128..+127)
                return wsb[nm][:, k * D + cols.start:k * D + cols.stop]

            def wslice_qk(nm, k, n):
                # wq_hi/wk_hi: by-n layout (col block n holds all k slices
                # of output block n) so projections for n=0 can start after
                # 1/6 of the weight DMA
                return wsb[nm][:, n * D + k * 128:n * D + (k + 1) * 128]

            xhi = [Ps.tile([128, R], f16, name=f"xhi{i}") for i in range(ND)]
            xlo = [Ps.tile([128, R], f16, name=f"xlo{i}") for i in range(ND)]
            ones_row = Ps.tile([128, CW], f16, name="ones_row")
            id32 = Ps.tile([128, 128], f32, name="id32")
            negmagic = Ps.tile([128, 1], f32, name="negmagic")

            nc.vector.memset(ones_row, 1.0)
            nc.vector.memset(negmagic, -MAGIC)
            # HAM warm-up: ~5us of dummy matmuls while the x/weight DMAs
            # are in flight, so the PE clock gate is at full rate (8/8)
            # by the time real work issues.  Results are never read.
            for _ in range(16):
                wt = ptile("warm")
                nc.tensor.matmul(wt, ones_row[:, :128], ones_row,
                                 start=True, stop=True)
            make_identity(nc, id32)
            if has_bias:
                bq_sb = Ps.tile([1, D], f16, name="bq_sb")
                bk_sb = Ps.tile([1, D], f16, name="bk_sb")
                bv_sb = Ps.tile([1, D], f16, name="bv_sb")
                bo_sb = Ps.tile([1, D], f16, name="bo_sb")

            # ---------------- pipeline pieces ----------------
            def load_split_rowchunk(rc):
                roff, rn = ROWCHUNKS[rc]
                xs = Pw.tile([128, D], f32, name="xs", tag="xs", bufs=4)
                nc.sync.dma_start(xs[:rn, :], x_d[roff:roff + rn, :])
                for d in range(ND):
                    tp = ptile("tp")
                    nc.tensor.transpose(tp[:128, :rn],
                                        xs[:rn, d * 128:(d + 1) * 128],
                                        id32[:rn, :rn])
                    hi = xhi[d][:, roff:roff + rn]
                    nc.vector.tensor_copy(hi, tp[:128, :rn])
                    nc.vector.tensor_tensor(xlo[d][:, roff:roff + rn],
                                            tp[:128, :rn], hi, OP.subtract)

            def proj_group(pre, n, c, dst_tiles):
                """One q/k projection group: 18 fp16 MMs -> qT/kT tile."""
                whi, wlo, b_nm = {"q": ("wq_hi", "wq_lo", "bq"),
                                  "k": ("wk_hi", "wk_lo", "bk")}[pre]
                cs = slice(c * CW, (c + 1) * CW)
                ns = slice(n * 128, (n + 1) * 128)
                # fp16 qT/kT: the scores matmuls then run at 1 cyc/row
                # (4x cheaper than fp32).  Emulated+HW-validated: adds
                # q/k fp16 rounding ahead of the floor(), total rel err
                # ~1.5e-2 vs the 2e-2 gate.
                dst = Pw.tile([128, CW], f16, name=f"{pre}T{n}",
                              tag=f"{pre}T{n}", bufs=2)
                pp = ptile("pp")
                for k in range(ND):
                    last = k == ND - 1 and not has_bias
                    nc.tensor.matmul(pp, wslice_qk(whi, k, n), xhi[k][:, cs],
                                     start=(k == 0), stop=False)
                    nc.tensor.matmul(pp, wslice_qk(whi, k, n), xlo[k][:, cs],
                                     start=False,
                                     stop=(last and not THREE_TERM))
                    if THREE_TERM:
                        nc.tensor.matmul(pp, wslice_qk(wlo, k, n),
                                         xhi[k][:, cs],
                                         start=False, stop=last)
                if has_bias:
                    bsb = {"bq": bq_sb, "bk": bk_sb}[b_nm]
                    nc.tensor.matmul(pp, bsb[:1, ns], ones_row[:1, :CW],
                                     start=False, stop=True)
                nc.scalar.activation(dst, pp, AF.Copy)
                dst_tiles[n] = dst

            def vproj_kc(b, kc, dst_tiles):
                """v projection for batch b, key chunk kc (fp16)."""
                base = b * T
                koff, klen = KEYCHUNKS[kc]
                dst = Pw.tile([128, H * HV], f16, name=f"v16e_{b}_{kc}",
                              tag="v16e", bufs=8)
                # only the per-head ones-columns need initializing; the
                # value region is fully overwritten by the copies below
                nc.vector.memset(
                    dst[:klen, :].rearrange(
                        "p (h c) -> p h c", c=HV)[:, :, DK:HV], 1.0)
                dst3 = dst[:klen, :].rearrange(
                    "p (h c) -> p h c", c=HV)[:, :, 0:DK]
                for half in range(2):
                    c0 = half * 384
                    vp = ptile("vp")
                    vps = vp[:klen, :384]
                    for d in range(ND):
                        nc.tensor.matmul(
                            vps, xhi[d][:, base + koff:base + koff + klen],
                            wslice("wv", d, slice(c0, c0 + 384)),
                            start=(d == 0),
                            stop=(d == ND - 1 and not has_bias))
                    if has_bias:
                        nc.tensor.matmul(vps, ones_row[:1, :klen],
                                         bv_sb[:1, c0:c0 + 384],
                                         start=False, stop=True)
                    nc.scalar.activation(
                        dst3[:, half * 6:(half + 1) * 6, :],
                        vps.rearrange("p (h c) -> p h c", c=DK),
                        AF.Copy)
                dst_tiles[kc] = dst

            def scores_hp(qk, bl, hp, eTs):
                """scoresT + floor + exp for one head-pair of local batch
                element bl (0/1) of the current chunk."""
                qT, kT = qk["q"], qk["k"]
                qs = slice(bl * T, (bl + 1) * T)
                eT = []
                for hl in range(2):
                    pb = 64 * hl
                    sc = ptile("sc")
                    for kc, (koff, klen) in enumerate(KEYCHUNKS):
                        ks = slice(bl * T + koff, bl * T + koff + klen)
                        nc.tensor.matmul(
                            sc[:klen, kc * T:(kc + 1) * T],
                            kT[hp][pb:pb + 64, ks],
                            qT[hp][pb:pb + 64, qs],
                            start=True, stop=True,
                            tile_position=(pb, 0))
                    fl = Pw.tile([128, 2 * T], f32, name="fl", tag="fl",
                                 bufs=5)
                    nc.vector.tensor_scalar(
                        fl, sc, -0.5, MAGIC, OP.add, OP.add)
                    e_t = Pw.tile([128, 2 * T], f16, name="e_t",
                                  tag="eT", bufs=24)
                    nc.scalar.activation(
                        e_t, fl, AF.Exp, bias=negmagic[:, :1])
                    eT.append(e_t)
                eTs[hp] = eT

            def av_hp(v16e, eTs, b, hp, st):
                """attn @ v for one head-pair; col 64 of v16e = ones ->
                denominators gathered into st['dn']."""
                if "dn" not in st:
                    st["dn"] = Pw.tile([128, 4 * T], f32, name="dn",
                                       tag="dn", bufs=2)
                    st["oT16"] = [None] * ND
                    st["otf"] = [None] * ND
                dn = st["dn"]
                otf = Pw.tile([128, T], f16, name="otf", tag="otf", bufs=13)
                op_ = ptile("oT")
                for hl in range(2):
                    h = 2 * hp + hl
                    for kc, (koff, klen) in enumerate(KEYCHUNKS):
                        nc.tensor.matmul(
                            op_[0:HV, hl * T:(hl + 1) * T],
                            v16e[kc][:klen, h * HV:(h + 1) * HV],
                            eTs[hp][hl][:klen, kc * T:(kc + 1) * T],
                            start=(kc == 0),
                            stop=(kc == len(KEYCHUNKS) - 1))
                    if hl == 0:
                        nc.scalar.activation(otf[0:64, :],
                                             op_[0:64, :T], AF.Copy)
                    else:
                        nc.vector.tensor_copy(otf[64:128, :],
                                              op_[0:64, T:2 * T])
                # denominators for both heads of the pair in one strip:
                # head-pair hp at partition 32*(hp%4), col block hp//4
                pbase = 32 * (hp % 4)
                cb = (hp // 4) * 2 * T
                nc.vector.tensor_copy(dn[pbase:pbase + 1, cb:cb + 2 * T],
                                      op_[64:65, :])
                st["otf"][hp] = otf

            def norm_recip(st):
                rdf = Pw.tile([128, 4 * T], f32, name="rdf", tag="rdf",
                              bufs=2)
                rd16 = Pw.tile([128, 4 * T], f16, name="rd16", tag="rd16",
                               bufs=2)
                nc.vector.reciprocal_approx_fast(rdf, st["dn"])
                nc.vector.tensor_copy(rd16, rdf)
                st["rd16"] = rd16

            def norm_hp(b, hp, st):
                rd16 = st["rd16"]
                oT16 = Pw.tile([128, T], f16, name=f"oT16_{b}_{hp}",
                               tag="oT16", bufs=26)
                bc = ptile("bc")
                pbase = 32 * (hp % 4)
                for hl in range(2):
                    cb = (hp // 4) * 2 * T + hl * T
                    nc.tensor.matmul(
                        bc[64 * hl:64 * hl + 64, :T],
                        ones_row[pbase:pbase + 1, :64],
                        rd16[pbase:pbase + 1, cb:cb + T],
                        start=True, stop=True,
                        tile_position=(pbase, 64 * hl))
                nc.vector.tensor_tensor(oT16, st["otf"][hp],
                                        bc[:, :T], OP.mult)
                st["oT16"][hp] = oT16

            def final_rc(b, rcl, st):
                """out-projection for token rows rcl of batch b + store."""
                base = b * T
                roff, rn = ((0, 128), (128, T - 128))[rcl]
                oT16 = st["oT16"]
                fs = Pw.tile([128, D], f32, name="fs", tag="fs", bufs=4)
                for half in range(2):
                    c0 = half * 384
                    fp_ = ptile("fp")
                    for d in range(ND):
                        nc.tensor.matmul(
                            fp_[:rn, :384],
                            oT16[d][:, roff:roff + rn],
                            wslice("wo", d, slice(c0, c0 + 384)),
                            start=(d == 0),
                            stop=(d == ND - 1 and not has_bias))
                    if has_bias:
                        nc.tensor.matmul(
                            fp_[:rn, :384], ones_row[:1, :rn],
                            bo_sb[:1, c0:c0 + 384],
                            start=False, stop=True)
                    nc.scalar.activation(fs[:rn, c0:c0 + 384],
                                         fp_[:rn, :384], AF.Copy)
                nc.sync.dma_start(
                    out_d[base + roff:base + roff + rn, :], fs[:rn, :])

            def chunk_rowchunks(c):
                lo = (CW * c) // 128
                hi = (CW * c + CW - 1) // 128
                return lo, hi

            def emit_interleaved(chain, filler):
                """Alternate chain units with filler units so the static
                per-engine order has dense PE work between dependent
                attention steps.  chain entries are (thunk, weight):
                weight = how much filler to place after the unit, scaled
                so all filler is spent.  The recip units carry extra
                weight to cover the serial DVE recip->cast chain before
                the broadcast matmuls."""
                if not chain:
                    for th in filler:
                        th()
                    return
                total_w = sum(w for _, w in chain) or 1.0
                scale = len(filler) / total_w
                acc, fi = 0.0, 0
                for th, w in chain:
                    th()
                    acc += w * scale
                    while fi < len(filler) and acc >= 1.0 - 1e-9:
                        filler[fi]()
                        fi += 1
                        acc -= 1.0
                while fi < len(filler):
                    filler[fi]()
                    fi += 1

            # ---------------- emission ----------------
            # prologue: x rowchunk DMAs+transposes for chunk 0 first, then
            # weight DMAs (q/k first), then chunk-0 projections + v.
            lo, hi = chunk_rowchunks(0)
            for rc in range(lo, hi + 1):
                load_split_rowchunk(rc)
            done_rc = hi
            whalf = ND * D // 2
            qk_first = [nm for nm in wnames if nm not in ("wv", "wo")]
            for nm in qk_first:
                nc.sync.dma_start(wsb[nm][:, :whalf], w_d[nm][:, :whalf])
            for nm in qk_first:
                nc.sync.dma_start(wsb[nm][:, whalf:], w_d[nm][:, whalf:])
            for nm in ("wv", "wo"):
                nc.sync.dma_start(wsb[nm][:, :whalf], w_d[nm][:, :whalf])
                nc.sync.dma_start(wsb[nm][:, whalf:], w_d[nm][:, whalf:])
            if has_bias:
                nc.sync.dma_start(bq_sb, bq_d)
                nc.sync.dma_start(bk_sb, bk_d)
                nc.sync.dma_start(bv_sb, bv_d)
                nc.sync.dma_start(bo_sb, bo_d)

            qk = {"q": [None] * ND, "k": [None] * ND}
            for n in range(ND):
                proj_group("q", n, 0, qk["q"])
                proj_group("k", n, 0, qk["k"])
            vt = {}
            for b in (0, 1):
                vt[b] = [None, None]
                vproj_kc(b, 0, vt[b])
                vproj_kc(b, 1, vt[b])

            deferred = []
            carry = []
            for c in range(NC4):
                b0, b1 = 2 * c, 2 * c + 1
                cur_qk, cur_v0, cur_v1 = qk, vt[b0], vt[b1]
                e0, e1 = [None] * ND, [None] * ND
                st0, st1 = {}, {}

                chain = []
                for hp in range(ND):
                    chain.append((lambda hp=hp: scores_hp(cur_qk, 0, hp, e0),
                                  1.0))
                    chain.append((lambda hp=hp: scores_hp(cur_qk, 1, hp, e1),
                                  1.0))
                # previous chunk's output projections are dependency-free
                # dense PE anchors: two placed mid-attn@v (which otherwise
                # paces behind ScalarE exp), two after the recips to cover
                # the serial DVE recip->cast chain
                for hp in range(ND):
                    chain.append((lambda hp=hp: av_hp(cur_v0, e0, b0, hp,
                                                      st0), 0.7))
                    chain.append((lambda hp=hp: av_hp(cur_v1, e1, b1, hp,
                                                      st1), 0.7))
                    if hp == 2 and len(carry) > 2:
                        chain.append((carry[0], 0.3))
                        chain.append((carry[1], 0.3))
                chain.append((lambda: norm_recip(st0), 3.0))
                chain.append((lambda: norm_recip(st1), 3.0))
                for th in carry[2:]:
                    chain.append((th, 0.3))
                for hp in range(ND):
                    chain.append((lambda hp=hp: norm_hp(b0, hp, st0), 0.8))
                    chain.append((lambda hp=hp: norm_hp(b1, hp, st1), 0.8))
                carry = []
                for rcl in range(2):
                    carry.append(lambda rcl=rcl, b=b0, st=st0:
                                 final_rc(b, rcl, st))
                    carry.append(lambda rcl=rcl, b=b1, st=st1:
                                 final_rc(b, rcl, st))
                if c == NC4 - 1:
                    for th in carry:
                        chain.append((th, 0.3))
                    carry = []

                filler = []
                if c + 1 < NC4:
                    lo, hi = chunk_rowchunks(c + 1)
                    for rc in range(done_rc + 1, hi + 1):
                        filler.append(
                            lambda rc=rc: load_split_rowchunk(rc))
                    done_rc = hi
                    nqk = {"q": [None] * ND, "k": [None] * ND}
                    for n in range(ND):
                        if c + 1 == NC4 - 1 and n == ND - 1:
                            # defer the last projection groups into the
                            # final (otherwise filler-less) chain; scores
                            # for head-pair 5 only run ~10 units in
                            deferred.append(
                                lambda n=n, c=c: proj_group("q", n, c + 1,
                                                            nqk["q"]))
                            deferred.append(
                                lambda n=n, c=c: proj_group("k", n, c + 1,
                                                            nqk["k"]))
                            continue
                        filler.append(
                            lambda n=n: proj_group("q", n, c + 1, nqk["q"]))
                        filler.append(
                            lambda n=n: proj_group("k", n, c + 1, nqk["k"]))
                    nvt = {}
                    for b in (2 * c + 2, 2 * c + 3):
                        nvt[b] = [None, None]
                        for kc in range(2):
                            if c + 1 == NC4 - 1 and b == 2 * c + 3:
                                # defer the last batch element's v proj to
                                # serve as PE filler inside the final
                                # (otherwise filler-less) chain
                                deferred.append(
                                    lambda b=b, kc=kc:
                                    vproj_kc(b, kc, nvt[b]))
                            else:
                                filler.append(
                                    lambda b=b, kc=kc:
                                    vproj_kc(b, kc, nvt[b]))
                    qk, vt = nqk, nvt

                if c == NC4 - 1 and deferred:
                    # hand-placed: deferred proj/v-proj units early in the
                    # chain (each completes before its consumer unit)
                    for i, (th, _) in enumerate(chain):
                        th()
                        if i in (1, 3, 5, 7) and (i - 1) // 2 < len(deferred):
                            deferred[(i - 1) // 2]()
                else:
                    emit_interleaved(chain, filler)

    nc.compile()
    return nc


def _split16(a):
    hi = a.astype(np.float16)
    lo = (a - hi.astype(np.float32)).astype(np.float16)
    return hi, lo


def _rearr(w16):
    """[768, 768] -> [128, 6*768]: col block k holds W rows k*128..+127."""
    return np.ascontiguousarray(
        w16.reshape(ND, 128, D).transpose(1, 0, 2).reshape(128, ND * D))


def _rearr_qk(w16):
    """[768, 768] -> [128, 6*768]: col block n holds all k-slices of
    output block n (dst[p, n*768+k*128+c] = W[k*128+p, n*128+c])."""
    return np.ascontiguousarray(
        w16.reshape(ND, 128, ND, 128).transpose(1, 2, 0, 3)
        .reshape(128, ND * D))


def _prep_weights(Wq, bq, Wk, bk, Wv, bv, Wo, bo, has_bias):
    f32 = np.float32
    wq = np.asarray(Wq, f32) * f32(0.125)
    wk = np.asarray(Wk, f32)
    wq_hi, wq_lo = _split16(wq)
    wk_hi, wk_lo = _split16(wk)
    w = {
        "wq_hi": _rearr_qk(wq_hi), "wk_hi": _rearr_qk(wk_hi),
        "wv": _rearr(np.asarray(Wv, f32).astype(np.float16)),
        "wo": _rearr(np.asarray(Wo, f32).astype(np.float16)),
    }
    if THREE_TERM:
        w["wq_lo"] = _rearr_qk(wq_lo)
        w["wk_lo"] = _rearr_qk(wk_lo)
    if has_bias:
        w["bq"] = (np.asarray(bq, f32) * f32(0.125)).astype(
            np.float16).reshape(1, D)
        w["bk"] = np.asarray(bk, f32).astype(np.float16).reshape(1, D)
        w["bv"] = np.asarray(bv, f32).astype(np.float16).reshape(1, D)
        w["bo"] = np.asarray(bo, f32).astype(np.float16).reshape(1, D)
    return w


def kernel(x, Wq, bq, Wk, bk, Wv, bv, Wo, bo):
    from concourse import bass_utils

    has_bias = any(float(np.abs(np.asarray(v)).max()) != 0.0
                   for v in (bq, bk, bv, bo))
    key = ("nc", has_bias)
    if key not in _CACHE:
        _CACHE[key] = _build(has_bias)
    nc = _CACHE[key]

    x = np.asarray(x, np.float32)
    w = _prep_weights(Wq, bq, Wk, bk, Wv, bv, Wo, bo, has_bias)
    in_maps = []
    for c in range(NCORES):
        m = dict(w)
        m["x"] = np.ascontiguousarray(
            x[c * BL:(c + 1) * BL].reshape(R, D))
        in_maps.append(m)

    res = bass_utils.run_bass_kernel_spmd(nc, in_maps, list(range(NCORES)))
    out = np.concatenate(
        [res.results[c]["out"].reshape(BL, T, D) for c in range(NCORES)],
        axis=0)
    return out.astype(np.float32)
